# revision 1
# baseline (speedup 1.0000x reference)
"""Trainium2 Bass kernel for the CAP loss (camera-aware proxy memory bank).

Strategy (8 NeuronCores, SPMD, raw Bass engine blocks):
  - The center bank [32000, 2048] is sharded along the center axis: 4000
    centers (= 500 labels x 8 cams, label-major) per core, pre-transposed and
    cast to bf16 on the host so each core streams a [2048, 4000] bf16 shard
    as 8 fully-contiguous 2MB slabs.
  - feats are replicated; the [256, 4000] similarity tile per core is computed
    as 2x8x16 PE matmuls (K=2048 accumulated in PSUM), exp applied on the
    scalar engine straight out of PSUM with a per-sample 1/(T*||f_i||) scale.
  - Because the bank is label-major with C=8 cams, every mask in the loss is a
    static stride pattern: intra-cam denominators are per-residue (mod 8)
    sums, the same-label sums are per-8-block sums, and the first-50
    hard-negative sum is a prefix over global columns [0,50)/[0,58) (core 0).
    All are strided vector-engine reductions - no gathers on device.
  - The own-logit numerator is a per-sample dot with its own center (host
    gathers the 256 own centers, 32 samples' worth per core).
  - The tiny [256]-sized tail (log, segment means over labels/cams) runs on
    the host at gather time.

Raw Bass (nc.Block) is used instead of the Tile framework: the installed
walrus rejects two raw-ISA instructions Tile's exit barrier emits
(EVENT_SEMAPHORE_RANGE_CLEAR, multi-wait DRAIN) and InstTensorTensorReduce.
"""

import numpy as np
import ml_dtypes
from contextlib import ExitStack

import concourse.bass as bass
from concourse import mybir
from concourse.bass_utils import run_bass_kernel_spmd

# problem constants (hardcoded per harness contract)
N, D, M = 256, 2048, 32000
L, C = 4000, 8
T = 0.07
LAMDA = 0.5
NCORES = 8
SHARD = M // NCORES          # 4000 centers per core
LBL_SHARD = SHARD // C       # 500 labels per core
CHUNK = 500                  # matmul moving free dim; 8 chunks per shard
NCHUNKS = SHARD // CHUNK     # 8
QUARTER = SHARD // 4         # 1000 cols = 125 whole label blocks
KT = D // 128                # 16 k-tiles
NS = N // NCORES             # 32 samples per core for the own-logit dot
NSLAB = 4                    # slab ring depth

F32 = mybir.dt.float32
BF16 = mybir.dt.bfloat16
ADD = mybir.AluOpType.add
AX = mybir.AxisListType.X
EXP = mybir.ActivationFunctionType.Exp


SQUARE = mybir.ActivationFunctionType.Square
F16 = mybir.dt.float16
NPSUM = 4                    # psum bank pairs: PE runs up to 4 chunks ahead of exp
NWARM = 24                   # dummy matmuls to warm the PE clock before chunk 0
W_FULL = 512                 # chunk width (64 whole labels, 0 mod 8)
W_LAST = SHARD - 7 * W_FULL  # 416 (52 whole labels)
CW = [W_FULL] * 7 + [W_LAST]
# layout of the consolidated small output [128, 2, 68] per m:
#   cols 8n+r (n<8, r<8) = per-chunk camera-residue exp sums (512 = 0 mod 8,
#       so chunk-local residue == global residue; host just sums chunks)
#   cols 64:66 = prefix sums P50, P58 (host uses core 0's)
#   col  66    = per-sample feat norm ||f_i||
#   col  67    = own-dot (raw <f_i, own_center_i>), rows 0:32 of m=0 only
SM_W = 68


def _build_program() -> bass.Bass:
    nc = bass.Bass()
    cTa = nc.dram_tensor("cTa", [7, 128, KT, W_FULL], BF16, kind="ExternalInput")
    cTb = nc.dram_tensor("cTb", [128, KT, W_LAST], BF16, kind="ExternalInput")
    fT = nc.dram_tensor("fT", [128, KT, N], BF16, kind="ExternalInput")
    fhd = nc.dram_tensor("feats16", [2, 128, D], F16, kind="ExternalInput")
    fsd = nc.dram_tensor("fs16", [NS, D], F16, kind="ExternalInput")
    ocd = nc.dram_tensor("oc16", [NS, D], F16, kind="ExternalInput")
    sm_out = nc.dram_tensor("SM_out", [128, 2, SM_W], F32, kind="ExternalOutput")
    bs_out = nc.dram_tensor("BS_out", [2, 128, LBL_SHARD], F32,
                            kind="ExternalOutput")

    with ExitStack() as ctx:
        e = ctx.enter_context

        ft_sb = e(nc.sbuf_tensor("ft_sb", [128, KT, N], BF16))
        slabs = [e(nc.sbuf_tensor(f"slab{j}", [128, KT, W_FULL], BF16))
                 for j in range(NSLAB)]
        et = [e(nc.sbuf_tensor(f"e{m}", [128, SHARD], F32)) for m in range(2)]
        fh_sb = e(nc.sbuf_tensor("fh_sb", [128, 2, D], F16))
        sq = e(nc.sbuf_tensor("sq", [128, D], F32))
        fs_sb = e(nc.sbuf_tensor("fs_sb", [NS, D], F16))
        oc_sb = e(nc.sbuf_tensor("oc_sb", [NS, D], F16))
        scr = e(nc.sbuf_tensor("scr", [NS, D], F32))

        ssum = [e(nc.sbuf_tensor(f"ssum{m}", [128, 1], F32)) for m in range(2)]
        inv = [e(nc.sbuf_tensor(f"inv{m}", [128, 1], F32)) for m in range(2)]
        sv = [e(nc.sbuf_tensor(f"sv{m}", [128, 1], F32)) for m in range(2)]

        bs = [e(nc.sbuf_tensor(f"bs{m}", [128, LBL_SHARD], F32)) for m in range(2)]
        small = e(nc.sbuf_tensor("small", [128, 2, SM_W], F32))

        ps = [[e(nc.psum_tensor(f"ps{b}_{m}", [128, W_FULL], F32))
               for m in range(2)] for b in range(NPSUM)]

        sem_ft = e(nc.semaphore("sem_ft"))
        sem_ftb = e(nc.semaphore("sem_ftb"))
        sem_slab = [e(nc.semaphore(f"sem_slab{j}")) for j in range(NSLAB)]
        sem_slab0b = e(nc.semaphore("sem_slab0b"))
        sem_f16 = e(nc.semaphore("sem_f16"))
        sem_fso = e(nc.semaphore("sem_fso"))
        sem_pe = e(nc.semaphore("sem_pe"))
        sem_act = e(nc.semaphore("sem_act"))
        c_a = e(nc.semaphore("c_a"))       # ACT prologue progress
        c_v = e(nc.semaphore("c_v"))       # DVE progress: every vector op incs
        c_warm = e(nc.semaphore("c_warm"))
        sem_od = e(nc.semaphore("sem_od"))

        # DVE instruction indices (c_v values after each op)
        V_SV = 5              # sv0 and sv1 both written
        V_P = 11              # dot + p50/58 done
        V_HALF = V_P + 4 * 4  # chunk 0..3 reductions done
        V_LAST = V_P + 8 * 4  # all chunk reductions done

        block = e(nc.Block(no_gpsimd_drain=True))

        @block.sync
        def _(sync):
            # first ft half, first slab0 half: minimal path to the first matmul
            sync.dma_start(out=ft_sb[:, 0:8, :], in_=fT[:, 0:8, :]).then_inc(
                sem_ft, 16)
            sync.dma_start(out=slabs[0][:, 0:8, :],
                           in_=cTa[0, :, 0:8, :]).then_inc(sem_slab[0], 16)
            sync.dma_start(out=ft_sb[:, 8:16, :], in_=fT[:, 8:16, :]).then_inc(
                sem_ftb, 16)
            sync.dma_start(out=slabs[0][:, 8:16, :],
                           in_=cTa[0, :, 8:16, :]).then_inc(sem_slab0b, 16)
            for n in range(1, NCHUNKS):
                j = n % NSLAB
                if n >= NSLAB:
                    # slot free once PE finished chunk n-NSLAB
                    sync.wait_ge(sem_pe, n - NSLAB + 1)
                if n < 7:
                    sync.dma_start(out=slabs[j][:, :, :], in_=cTa[n]).then_inc(
                        sem_slab[j], 16)
                else:
                    sync.dma_start(out=slabs[j][:, :, 0:W_LAST],
                                   in_=cTb[:, :, :]).then_inc(sem_slab[j], 16)
            # early writeback of the first four chunks' label-block sums
            sync.wait_ge(c_v, V_HALF)
            sync.dma_start(out=bs_out[0][:, 0:256], in_=bs[0][:, 0:256]).then_inc(
                sem_od, 16)
            sync.dma_start(out=bs_out[1][:, 0:256], in_=bs[1][:, 0:256]).then_inc(
                sem_od, 16)
            # final writeback
            sync.wait_ge(c_v, V_LAST)
            sync.dma_start(out=sm_out[:, :, :], in_=small[:, :, :]).then_inc(
                sem_od, 16)
            sync.dma_start(out=bs_out[0][:, 256:500],
                           in_=bs[0][:, 256:500]).then_inc(sem_od, 16)
            sync.dma_start(out=bs_out[1][:, 256:500],
                           in_=bs[1][:, 256:500]).then_inc(sem_od, 16)
            sync.wait_ge(sem_od, 80)

        @block.tensor
        def _(tensor):
            tensor.wait_ge(sem_ft, 16)
            # dummy matmuls on the already-loaded ft half: warms the PE clock
            # gate (HAM) while the first center slab is still in flight
            last = None
            for w in range(NWARM):
                last = tensor.matmul(ps[NPSUM - 1][0][:, 0:N],
                                     ft_sb[:, 0, 0:128], ft_sb[:, 0, :],
                                     start=True, stop=True)
            last.then_inc(c_warm, 1)
            slot_seen = [0] * NSLAB
            for n in range(NCHUNKS):
                j = n % NSLAB
                b = n % NPSUM
                w = CW[n]
                if n == 0:
                    tensor.wait_ge(sem_slab[0], 16)   # first half only
                    slot_seen[0] = 16
                else:
                    slot_seen[j] += 16
                    tensor.wait_ge(sem_slab[j], slot_seen[j])
                if n >= NPSUM:
                    # psum bank pair free once ACT consumed chunk n-NPSUM
                    tensor.wait_ge(sem_act, 2 * (n - NPSUM + 1))
                if n == NPSUM - 1:
                    # warmup dummies wrote this psum bank (WAW ordering)
                    tensor.wait_ge(c_warm, 1)
                last = None
                for ki in range(KT):
                    if n == 0 and ki == 8:
                        tensor.wait_ge(sem_ftb, 16)
                        tensor.wait_ge(sem_slab0b, 16)
                    for m in range(2):
                        last = tensor.matmul(
                            ps[b][m][:, 0:w],
                            ft_sb[:, ki, m * 128:(m + 1) * 128],
                            slabs[j][:, ki, 0:w],
                            start=(ki == 0), stop=(ki == KT - 1))
                last.then_inc(sem_pe, 1)

        @block.scalar
        def _(scalar):
            # setup inputs ride the ACT engine's own HW-DGE ring, in parallel
            # with the sync ring's ft/slab stream
            scalar.dma_start(
                out=fh_sb[:, :, :],
                in_=fhd.rearrange("m p d -> p m d")).then_inc(sem_f16, 16)
            scalar.dma_start(out=fs_sb[:, :], in_=fsd[:, :]).then_inc(sem_fso, 16)
            scalar.dma_start(out=oc_sb[:, :], in_=ocd[:, :]).then_inc(sem_fso, 16)
            # row sums-of-squares + norms for the exp scale (ACT-only prologue)
            scalar.wait_ge(sem_f16, 16)
            for m in range(2):
                scalar.activation(out=sq[:, :], in_=fh_sb[:, m, :], func=SQUARE,
                                  accum_out=ssum[m][:, :]).then_inc(c_a, 1)
                scalar.wait_ge(c_a, 2 * m + 1)
                scalar.sqrt(small[:, m, 66:67], ssum[m][:, :]).then_inc(c_a, 1)
            # exp stream straight out of PSUM with per-sample scale
            scalar.wait_ge(c_v, V_SV)
            for n in range(NCHUNKS):
                b = n % NPSUM
                w = CW[n]
                scalar.wait_ge(sem_pe, n + 1)
                for m in range(2):
                    scalar.activation(
                        out=et[m][:, n * W_FULL:n * W_FULL + w],
                        in_=ps[b][m][:, 0:w],
                        func=EXP, scale=sv[m][:, :]).then_inc(sem_act, 1)

        @block.vector
        def _(vector):
            vcount = 0

            def v(instr):
                nonlocal vcount
                instr.then_inc(c_v, 1)
                vcount += 1
                return vcount

            # zero the never-fully-written column of `small` (DMA'd out whole);
            # the dot-reduce overwrites rows 0:32 of m=0 later, in order
            v(vector.memset(small[:, :, 67:68], 0.0))              # op 1
            for m in range(2):                                     # ops 2..5
                vector.wait_ge(c_a, 2 * (m + 1))
                v(vector.reciprocal(inv[m][:, :], small[:, m, 66:67]))
                vector.wait_ge(c_v, vcount)
                v(vector.tensor_scalar_mul(sv[m][:, :], inv[m][:, :], 1.0 / T))
            assert vcount == V_SV
            # raw own-logit dot (host divides by T*norm at gather time)
            vector.wait_ge(sem_fso, 32)
            v(vector.tensor_mul(scr[:, :], fs_sb[:, :], oc_sb[:, :]))   # 6
            vector.wait_ge(c_v, vcount)
            v(vector.tensor_reduce(out=small[0:NS, 0, 67:68], in_=scr[:, :],  # 7
                                   axis=AX, op=ADD))
            # prefix sums over global columns [0,50)/[0,58) (host uses core 0's)
            vector.wait_ge(sem_act, 2)
            for m in range(2):                                     # ops 8..11
                v(vector.tensor_reduce(out=small[:, m, 64:65], in_=et[m][:, 0:50],
                                       axis=AX, op=ADD))
                v(vector.tensor_reduce(out=small[:, m, 65:66], in_=et[m][:, 0:58],
                                       axis=AX, op=ADD))
            assert vcount == V_P
            # per-chunk reductions right behind each exp: label-block sums and
            # camera-residue sums (chunks are 0 mod 8 wide -> fully aligned)
            for n in range(NCHUNKS):                               # 4 ops/chunk
                w = CW[n]
                nl = w // C                                        # 64 or 52
                vector.wait_ge(sem_act, 2 * (n + 1))
                for m in range(2):
                    chunk = et[m][:, n * W_FULL:n * W_FULL + w]
                    v(vector.tensor_reduce(
                        out=bs[m][:, 64 * n:64 * n + nl],
                        in_=chunk.rearrange("p (l r) -> p l r", r=C),
                        axis=AX, op=ADD))
                    v(vector.tensor_reduce(
                        out=small[:, m, 8 * n:8 * n + 8],
                        in_=chunk.rearrange("p (l r) -> p r l", r=C),
                        axis=AX, op=ADD))
            assert vcount == V_LAST

    return nc


_PROGRAM_CACHE: dict[str, bass.Bass] = {}


def _program() -> bass.Bass:
    if "nc" not in _PROGRAM_CACHE:
        _PROGRAM_CACHE["nc"] = _build_program()
    return _PROGRAM_CACHE["nc"]


def _make_in_maps(feats, centers, own_centers):
    bf = ml_dtypes.bfloat16
    fT_host = np.ascontiguousarray(feats.T)            # [2048, 256] f32
    fT_bf = fT_host.astype(bf).reshape(KT, 128, N).transpose(1, 0, 2)
    fT_bf = np.ascontiguousarray(fT_bf)                # [128, 16, 256]
    fh_host = feats.astype(np.float16).reshape(2, 128, D)
    cT_all = np.ascontiguousarray(centers.T).astype(bf)  # [2048, 32000] bf16

    in_maps = []
    for c in range(NCORES):
        shard = cT_all[:, c * SHARD:(c + 1) * SHARD]     # [2048, 4000]
        sk = shard.reshape(KT, 128, SHARD)               # [16, 128, 4000]
        a = sk[:, :, 0:7 * W_FULL].reshape(KT, 128, 7, W_FULL)
        a = np.ascontiguousarray(a.transpose(2, 1, 0, 3))  # [7, 128, 16, 512]
        b = np.ascontiguousarray(
            sk[:, :, 7 * W_FULL:].transpose(1, 0, 2))      # [128, 16, 416]
        in_maps.append({
            "cTa": a,
            "cTb": b,
            "fT": fT_bf,
            "feats16": fh_host,
            "fs16": np.ascontiguousarray(
                feats[c * NS:(c + 1) * NS].astype(np.float16)),
            "oc16": np.ascontiguousarray(
                own_centers[c * NS:(c + 1) * NS].astype(np.float16)),
        })
    return in_maps


def _host_tail(results, labels, camids, epoch):
    n = labels.shape[0]
    # SM_out [128, 2, SM_W]: sample i lives at [i % 128, i // 128, :]
    SM = [r["SM_out"].transpose(1, 0, 2).reshape(n, SM_W) for r in results]
    # per-chunk camera-residue sums (aligned: just sum over chunks and cores)
    S = np.zeros((n, C), np.float32)
    for sm in SM:
        S += sm[:, 0:64].reshape(n, NCHUNKS, C).sum(axis=1)
    denom_intra = S[np.arange(n), camids]

    owner = (labels // LBL_SHARD).astype(np.int64)
    BS = np.stack([r["BS_out"].reshape(n, LBL_SHARD) for r in results])
    B = BS[owner, np.arange(n), labels % LBL_SHARD]
    p50, p58 = SM[0][:, 64], SM[0][:, 65]
    hard = np.where(labels <= 6, p58 - B, p50)
    denom_inter = B + hard

    nrm = SM[0][:, 66]                                # replicated across cores
    dot = np.concatenate([r["SM_out"][0:NS, 0, 67] for r in results])  # [n]
    own = dot / (T * nrm)

    loss_i = own - np.log(denom_intra)
    loss_j = own - np.log(denom_inter)

    cam_sums = np.zeros(C, np.float32)
    cam_cnts = np.zeros(C, np.float32)
    np.add.at(cam_sums, camids, loss_i)
    np.add.at(cam_cnts, camids, 1.0)
    loss_intra = -np.sum(
        np.where(cam_cnts > 0, cam_sums / np.maximum(cam_cnts, 1.0), 0.0),
        dtype=np.float32)

    lbl_sums = np.zeros(L, np.float32)
    lbl_cnts = np.zeros(L, np.float32)
    np.add.at(lbl_sums, labels, loss_j)
    np.add.at(lbl_cnts, labels, 1.0)
    loss_inter = -np.sum(
        np.where(lbl_cnts > 0, lbl_sums / np.maximum(lbl_cnts, 1.0), 0.0),
        dtype=np.float32)

    if int(epoch) < 5:
        return np.float32(loss_intra)
    return np.stack([loss_intra, LAMDA * loss_inter]).astype(np.float32)


def kernel(feats, centers, labels, camids, epoch):
    feats = np.ascontiguousarray(np.asarray(feats, dtype=np.float32))
    centers = np.ascontiguousarray(np.asarray(centers, dtype=np.float32))
    labels = np.asarray(labels).astype(np.int64)
    camids = np.asarray(camids).astype(np.int64)

    own_idx = labels * C + camids
    own_centers = centers[own_idx]                     # host gather [256, 2048]

    in_maps = _make_in_maps(feats, centers, own_centers)
    res = run_bass_kernel_spmd(_program(), in_maps, list(range(NCORES))).results
    return _host_tail(res, labels, camids, epoch)



# revision 7
# speedup vs baseline: 1.7003x; 1.7003x over previous
"""Trainium2 Bass kernel for the CAP loss (camera-aware proxy memory bank).

Strategy (8 NeuronCores, SPMD, raw Bass engine blocks):
  - The center bank [32000, 2048] is sharded along the center axis: 4000
    centers (= 500 labels x 8 cams, label-major) per core, pre-transposed and
    cast to bf16 on the host so each core streams a [2048, 4000] bf16 shard
    as 8 fully-contiguous 2MB slabs.
  - feats are replicated; the [256, 4000] similarity tile per core is computed
    as 2x8x16 PE matmuls (K=2048 accumulated in PSUM), exp applied on the
    scalar engine straight out of PSUM with a per-sample 1/(T*||f_i||) scale.
  - Because the bank is label-major with C=8 cams, every mask in the loss is a
    static stride pattern: intra-cam denominators are per-residue (mod 8)
    sums, the same-label sums are per-8-block sums, and the first-50
    hard-negative sum is a prefix over global columns [0,50)/[0,58) (core 0).
    All are strided vector-engine reductions - no gathers on device.
  - The own-logit numerator is a per-sample dot with its own center (host
    gathers the 256 own centers, 32 samples' worth per core).
  - The tiny [256]-sized tail (log, segment means over labels/cams) runs on
    the host at gather time.

Raw Bass (nc.Block) is used instead of the Tile framework: the installed
walrus rejects two raw-ISA instructions Tile's exit barrier emits
(EVENT_SEMAPHORE_RANGE_CLEAR, multi-wait DRAIN) and InstTensorTensorReduce.
"""

import numpy as np
import ml_dtypes
from contextlib import ExitStack

import concourse.bass as bass
from concourse import mybir
from concourse.bass_utils import run_bass_kernel_spmd

# problem constants (hardcoded per harness contract)
N, D, M = 256, 2048, 32000
L, C = 4000, 8
T = 0.07
LAMDA = 0.5
NCORES = 8
SHARD = M // NCORES          # 4000 centers per core
LBL_SHARD = SHARD // C       # 500 labels per core
CHUNK = 500                  # matmul moving free dim; 8 chunks per shard
NCHUNKS = SHARD // CHUNK     # 8
QUARTER = SHARD // 4         # 1000 cols = 125 whole label blocks
KT = D // 128                # 16 k-tiles
NS = N // NCORES             # 32 samples per core for the own-logit dot
NSLAB = 4                    # slab ring depth

F32 = mybir.dt.float32
BF16 = mybir.dt.bfloat16
FP8 = mybir.dt.float8e4
DR = mybir.MatmulPerfMode.DoubleRow
CSCALE = 32.0                # host scales centers by 32 before fp8 cast
ADD = mybir.AluOpType.add
AX = mybir.AxisListType.X
EXP = mybir.ActivationFunctionType.Exp


SQUARE = mybir.ActivationFunctionType.Square
F16 = mybir.dt.float16
NPSUM = 4                    # psum bank pairs: PE runs up to 4 chunks ahead of exp
NWARM = 24                   # dummy matmuls to warm the PE clock before chunk 0
W_FULL = 512                 # chunk width (64 whole labels, 0 mod 8)
W_LAST = SHARD - 7 * W_FULL  # 416 (52 whole labels)
CW = [W_FULL] * 7 + [W_LAST]
# layout of the consolidated small output [128, 2, 68] per m:
#   cols 8n+r (n<8, r<8) = per-chunk camera-residue exp sums (512 = 0 mod 8,
#       so chunk-local residue == global residue; host just sums chunks)
#   cols 64:66 = prefix sums P50, P58 (host uses core 0's)
#   col  66    = per-sample feat norm ||f_i||
#   col  67    = own-dot (raw <f_i, own_center_i>), rows 0:32 of m=0 only
SM_W = 68


def _build_program() -> bass.Bass:
    nc = bass.Bass()
    cTa = nc.dram_tensor("cTa", [7, 128, KT, W_FULL], FP8, kind="ExternalInput")
    cTb = nc.dram_tensor("cTb", [128, KT, W_LAST], FP8, kind="ExternalInput")
    fT = nc.dram_tensor("fT", [128, KT, N], FP8, kind="ExternalInput")
    fhd = nc.dram_tensor("feats16", [2, 128, D], F16, kind="ExternalInput")
    fsd = nc.dram_tensor("fs16", [NS, D], F16, kind="ExternalInput")
    ocd = nc.dram_tensor("oc16", [NS, D], F16, kind="ExternalInput")
    sm_out = nc.dram_tensor("SM_out", [128, 2, SM_W], F32, kind="ExternalOutput")
    bs_out = nc.dram_tensor("BS_out", [2, 128, LBL_SHARD], F32,
                            kind="ExternalOutput")

    with ExitStack() as ctx:
        e = ctx.enter_context

        ft_sb = e(nc.sbuf_tensor("ft_sb", [128, KT, N], FP8))
        slabs = [e(nc.sbuf_tensor(f"slab{j}", [128, KT, W_FULL], FP8))
                 for j in range(NSLAB)]
        et = [e(nc.sbuf_tensor(f"e{m}", [128, SHARD], F32)) for m in range(2)]
        fh_sb = e(nc.sbuf_tensor("fh_sb", [128, 2, D], F16))
        sq = e(nc.sbuf_tensor("sq", [128, D], F32))
        fs_sb = e(nc.sbuf_tensor("fs_sb", [NS, D], F16))
        oc_sb = e(nc.sbuf_tensor("oc_sb", [NS, D], F16))
        scr = e(nc.sbuf_tensor("scr", [NS, D], F32))

        ssum = [e(nc.sbuf_tensor(f"ssum{m}", [128, 1], F32)) for m in range(2)]
        inv = [e(nc.sbuf_tensor(f"inv{m}", [128, 1], F32)) for m in range(2)]
        sv = [e(nc.sbuf_tensor(f"sv{m}", [128, 1], F32)) for m in range(2)]

        bs = [e(nc.sbuf_tensor(f"bs{m}", [128, LBL_SHARD], F32)) for m in range(2)]
        small = e(nc.sbuf_tensor("small", [128, 2, SM_W], F32))

        ps = [[e(nc.psum_tensor(f"ps{b}_{m}", [128, W_FULL], F32))
               for m in range(2)] for b in range(NPSUM)]

        sem_ft = e(nc.semaphore("sem_ft"))
        sem_ftb = e(nc.semaphore("sem_ftb"))
        sem_slab = [e(nc.semaphore(f"sem_slab{j}")) for j in range(NSLAB)]
        sem_slab0b = e(nc.semaphore("sem_slab0b"))
        sem_f16 = e(nc.semaphore("sem_f16"))
        sem_fso = e(nc.semaphore("sem_fso"))
        sem_pe = e(nc.semaphore("sem_pe"))
        sem_act = e(nc.semaphore("sem_act"))
        c_a = e(nc.semaphore("c_a"))       # ACT prologue progress
        c_v = e(nc.semaphore("c_v"))       # DVE progress: every vector op incs
        c_warm = e(nc.semaphore("c_warm"))
        sem_od = e(nc.semaphore("sem_od"))

        # DVE instruction indices (c_v values after each op)
        V_SV = 5              # sv0 and sv1 both written
        V_P = 11              # dot + p50/58 done
        V_HALF = V_P + 4 * 4  # chunk 0..3 reductions done
        V_LAST = V_P + 8 * 4  # all chunk reductions done

        block = e(nc.Block(no_gpsimd_drain=True))

        @block.sync
        def _(sync):
            # first ft half, first slab0 half: minimal path to the first matmul
            sync.dma_start(out=ft_sb[:, 0:8, :], in_=fT[:, 0:8, :]).then_inc(
                sem_ft, 16)
            sync.dma_start(out=slabs[0][:, 0:8, :],
                           in_=cTa[0, :, 0:8, :]).then_inc(sem_slab[0], 16)
            sync.dma_start(out=ft_sb[:, 8:16, :], in_=fT[:, 8:16, :]).then_inc(
                sem_ftb, 16)
            sync.dma_start(out=slabs[0][:, 8:16, :],
                           in_=cTa[0, :, 8:16, :]).then_inc(sem_slab0b, 16)
            for n in range(1, NCHUNKS):
                j = n % NSLAB
                if n >= NSLAB:
                    # slot free once PE finished chunk n-NSLAB
                    sync.wait_ge(sem_pe, n - NSLAB + 1)
                if n < 7:
                    sync.dma_start(out=slabs[j][:, :, :], in_=cTa[n]).then_inc(
                        sem_slab[j], 16)
                else:
                    sync.dma_start(out=slabs[j][:, :, 0:W_LAST],
                                   in_=cTb[:, :, :]).then_inc(sem_slab[j], 16)
            # early writeback of the first four chunks' label-block sums
            sync.wait_ge(c_v, V_HALF)
            sync.dma_start(out=bs_out[0][:, 0:256], in_=bs[0][:, 0:256]).then_inc(
                sem_od, 16)
            sync.dma_start(out=bs_out[1][:, 0:256], in_=bs[1][:, 0:256]).then_inc(
                sem_od, 16)
            # final writeback
            sync.wait_ge(c_v, V_LAST)
            sync.dma_start(out=sm_out[:, :, :], in_=small[:, :, :]).then_inc(
                sem_od, 16)
            sync.dma_start(out=bs_out[0][:, 256:500],
                           in_=bs[0][:, 256:500]).then_inc(sem_od, 16)
            sync.dma_start(out=bs_out[1][:, 256:500],
                           in_=bs[1][:, 256:500]).then_inc(sem_od, 16)
            sync.wait_ge(sem_od, 80)

        @block.tensor
        def _(tensor):
            tensor.wait_ge(sem_ft, 16)
            # dummy matmuls on the already-loaded ft half: warms the PE clock
            # gate (HAM) while the first center slab is still in flight
            last = None
            for w in range(NWARM):
                last = tensor.matmul(ps[NPSUM - 1][0][:, 0:N],
                                     ft_sb[:, 0, 0:128], ft_sb[:, 0, :],
                                     start=True, stop=True)
            last.then_inc(c_warm, 1)
            slot_seen = [0] * NSLAB
            for n in range(NCHUNKS):
                j = n % NSLAB
                b = n % NPSUM
                w = CW[n]
                if n == 0:
                    tensor.wait_ge(sem_slab[0], 16)   # first half only
                    slot_seen[0] = 16
                else:
                    slot_seen[j] += 16
                    tensor.wait_ge(sem_slab[j], slot_seen[j])
                if n >= NPSUM:
                    # psum bank pair free once ACT consumed chunk n-NPSUM
                    tensor.wait_ge(sem_act, 2 * (n - NPSUM + 1))
                if n == NPSUM - 1:
                    # warmup dummies wrote this psum bank (WAW ordering)
                    tensor.wait_ge(c_warm, 1)
                last = None
                for ki in range(0, KT, 2):
                    if n == 0 and ki == 8:
                        tensor.wait_ge(sem_ftb, 16)
                        tensor.wait_ge(sem_slab0b, 16)
                    for m in range(2):
                        last = tensor.matmul(
                            ps[b][m][:, 0:w],
                            ft_sb[:, ki:ki + 2, m * 128:(m + 1) * 128],
                            slabs[j][:, ki:ki + 2, 0:w],
                            start=(ki == 0), stop=(ki == KT - 2),
                            perf_mode=DR)
                last.then_inc(sem_pe, 1)

        @block.scalar
        def _(scalar):
            # setup inputs ride the ACT engine's own HW-DGE ring, in parallel
            # with the sync ring's ft/slab stream
            scalar.dma_start(
                out=fh_sb[:, :, :],
                in_=fhd.rearrange("m p d -> p m d")).then_inc(sem_f16, 16)
            scalar.dma_start(out=fs_sb[:, :], in_=fsd[:, :]).then_inc(sem_fso, 16)
            scalar.dma_start(out=oc_sb[:, :], in_=ocd[:, :]).then_inc(sem_fso, 16)
            # row sums-of-squares + norms for the exp scale (ACT-only prologue)
            scalar.wait_ge(sem_f16, 16)
            for m in range(2):
                scalar.activation(out=sq[:, :], in_=fh_sb[:, m, :], func=SQUARE,
                                  accum_out=ssum[m][:, :]).then_inc(c_a, 1)
                scalar.wait_ge(c_a, 2 * m + 1)
                scalar.sqrt(small[:, m, 66:67], ssum[m][:, :]).then_inc(c_a, 1)
            # exp stream straight out of PSUM with per-sample scale
            scalar.wait_ge(c_v, V_SV)
            for n in range(NCHUNKS):
                b = n % NPSUM
                w = CW[n]
                scalar.wait_ge(sem_pe, n + 1)
                for m in range(2):
                    scalar.activation(
                        out=et[m][:, n * W_FULL:n * W_FULL + w],
                        in_=ps[b][m][:, 0:w],
                        func=EXP, scale=sv[m][:, :]).then_inc(sem_act, 1)

        @block.vector
        def _(vector):
            vcount = 0

            def v(instr):
                nonlocal vcount
                instr.then_inc(c_v, 1)
                vcount += 1
                return vcount

            # zero the never-fully-written column of `small` (DMA'd out whole);
            # the dot-reduce overwrites rows 0:32 of m=0 later, in order
            v(vector.memset(small[:, :, 67:68], 0.0))              # op 1
            for m in range(2):                                     # ops 2..5
                vector.wait_ge(c_a, 2 * (m + 1))
                v(vector.reciprocal(inv[m][:, :], small[:, m, 66:67]))
                vector.wait_ge(c_v, vcount)
                v(vector.tensor_scalar_mul(sv[m][:, :], inv[m][:, :],
                                           1.0 / (T * CSCALE)))
            assert vcount == V_SV
            # raw own-logit dot (host divides by T*norm at gather time)
            vector.wait_ge(sem_fso, 32)
            v(vector.tensor_mul(scr[:, :], fs_sb[:, :], oc_sb[:, :]))   # 6
            vector.wait_ge(c_v, vcount)
            v(vector.tensor_reduce(out=small[0:NS, 0, 67:68], in_=scr[:, :],  # 7
                                   axis=AX, op=ADD))
            # prefix sums over global columns [0,50)/[0,58) (host uses core 0's)
            vector.wait_ge(sem_act, 2)
            for m in range(2):                                     # ops 8..11
                v(vector.tensor_reduce(out=small[:, m, 64:65], in_=et[m][:, 0:50],
                                       axis=AX, op=ADD))
                v(vector.tensor_reduce(out=small[:, m, 65:66], in_=et[m][:, 0:58],
                                       axis=AX, op=ADD))
            assert vcount == V_P
            # per-chunk reductions right behind each exp: label-block sums and
            # camera-residue sums (chunks are 0 mod 8 wide -> fully aligned)
            for n in range(NCHUNKS):                               # 4 ops/chunk
                w = CW[n]
                nl = w // C                                        # 64 or 52
                vector.wait_ge(sem_act, 2 * (n + 1))
                for m in range(2):
                    chunk = et[m][:, n * W_FULL:n * W_FULL + w]
                    v(vector.tensor_reduce(
                        out=bs[m][:, 64 * n:64 * n + nl],
                        in_=chunk.rearrange("p (l r) -> p l r", r=C),
                        axis=AX, op=ADD))
                    v(vector.tensor_reduce(
                        out=small[:, m, 8 * n:8 * n + 8],
                        in_=chunk.rearrange("p (l r) -> p r l", r=C),
                        axis=AX, op=ADD))
            assert vcount == V_LAST

    return nc


_PROGRAM_CACHE: dict[str, bass.Bass] = {}


def _program() -> bass.Bass:
    if "nc" not in _PROGRAM_CACHE:
        _PROGRAM_CACHE["nc"] = _build_program()
    return _PROGRAM_CACHE["nc"]


def _make_in_maps(feats, centers, own_centers):
    f8 = ml_dtypes.float8_e4m3
    fT_host = np.ascontiguousarray(feats.T)            # [2048, 256] f32
    fT_bf = fT_host.astype(f8).reshape(KT, 128, N).transpose(1, 0, 2)
    fT_bf = np.ascontiguousarray(fT_bf)                # [128, 16, 256]
    fh_host = feats.astype(np.float16).reshape(2, 128, D)
    cT_all = (np.ascontiguousarray(centers.T) * CSCALE).astype(f8)

    in_maps = []
    for c in range(NCORES):
        shard = cT_all[:, c * SHARD:(c + 1) * SHARD]     # [2048, 4000]
        sk = shard.reshape(KT, 128, SHARD)               # [16, 128, 4000]
        a = sk[:, :, 0:7 * W_FULL].reshape(KT, 128, 7, W_FULL)
        a = np.ascontiguousarray(a.transpose(2, 1, 0, 3))  # [7, 128, 16, 512]
        b = np.ascontiguousarray(
            sk[:, :, 7 * W_FULL:].transpose(1, 0, 2))      # [128, 16, 416]
        in_maps.append({
            "cTa": a,
            "cTb": b,
            "fT": fT_bf,
            "feats16": fh_host,
            "fs16": np.ascontiguousarray(
                feats[c * NS:(c + 1) * NS].astype(np.float16)),
            "oc16": np.ascontiguousarray(
                own_centers[c * NS:(c + 1) * NS].astype(np.float16)),
        })
    return in_maps


def _host_tail(results, labels, camids, epoch):
    n = labels.shape[0]
    # SM_out [128, 2, SM_W]: sample i lives at [i % 128, i // 128, :]
    SM = [r["SM_out"].transpose(1, 0, 2).reshape(n, SM_W) for r in results]
    # per-chunk camera-residue sums (aligned: just sum over chunks and cores)
    S = np.zeros((n, C), np.float32)
    for sm in SM:
        S += sm[:, 0:64].reshape(n, NCHUNKS, C).sum(axis=1)
    denom_intra = S[np.arange(n), camids]

    owner = (labels // LBL_SHARD).astype(np.int64)
    BS = np.stack([r["BS_out"].reshape(n, LBL_SHARD) for r in results])
    B = BS[owner, np.arange(n), labels % LBL_SHARD]
    p50, p58 = SM[0][:, 64], SM[0][:, 65]
    hard = np.where(labels <= 6, p58 - B, p50)
    denom_inter = B + hard

    nrm = SM[0][:, 66]                                # replicated across cores
    dot = np.concatenate([r["SM_out"][0:NS, 0, 67] for r in results])  # [n]
    own = dot / (T * nrm)

    loss_i = own - np.log(denom_intra)
    loss_j = own - np.log(denom_inter)

    cam_sums = np.zeros(C, np.float32)
    cam_cnts = np.zeros(C, np.float32)
    np.add.at(cam_sums, camids, loss_i)
    np.add.at(cam_cnts, camids, 1.0)
    loss_intra = -np.sum(
        np.where(cam_cnts > 0, cam_sums / np.maximum(cam_cnts, 1.0), 0.0),
        dtype=np.float32)

    lbl_sums = np.zeros(L, np.float32)
    lbl_cnts = np.zeros(L, np.float32)
    np.add.at(lbl_sums, labels, loss_j)
    np.add.at(lbl_cnts, labels, 1.0)
    loss_inter = -np.sum(
        np.where(lbl_cnts > 0, lbl_sums / np.maximum(lbl_cnts, 1.0), 0.0),
        dtype=np.float32)

    if int(epoch) < 5:
        return np.float32(loss_intra)
    return np.stack([loss_intra, LAMDA * loss_inter]).astype(np.float32)


def kernel(feats, centers, labels, camids, epoch):
    feats = np.ascontiguousarray(np.asarray(feats, dtype=np.float32))
    centers = np.ascontiguousarray(np.asarray(centers, dtype=np.float32))
    labels = np.asarray(labels).astype(np.int64)
    camids = np.asarray(camids).astype(np.int64)

    own_idx = labels * C + camids
    own_centers = centers[own_idx]                     # host gather [256, 2048]

    in_maps = _make_in_maps(feats, centers, own_centers)
    res = run_bass_kernel_spmd(_program(), in_maps, list(range(NCORES))).results
    return _host_tail(res, labels, camids, epoch)



# revision 14
# speedup vs baseline: 2.2544x; 1.3258x over previous
"""Trainium2 Bass kernel for the CAP loss (camera-aware proxy memory bank).

Strategy (8 NeuronCores, SPMD, raw Bass engine blocks):
  - The center bank [32000, 2048] is sharded along the center axis (4000
    centers per core) and reordered cam-major on the host: each core holds
    8 slabs of 500 columns (one slab per camera), fp8(e4m3), scaled x32,
    pre-transposed to [128, 16, 512] (cols padded 500->512 for the
    DoubleRow k-pair stride requirement).
  - Samples are sorted by camid on the host; feats are replicated (fp8).
    Per slab g the PE computes only the rows of camera g (DoubleRow fp8
    matmuls, K=2048 accumulated in PSUM) - the intra-camera mask reduces
    useful compute 8x vs the dense [256 x 4000] product. Outputs land at
    PSUM partition base 0; the per-sample exp scale and the accumulator
    are laid out per piece (one column each), so no partition alignment
    with the sample index is needed.
  - The ACT engine applies exp straight out of PSUM with a per-sample
    1/(32*T*||f8||) scale and its fused accum_out produces the per-sample
    partial intra denominators directly. No vector-engine work at all.
  - Everything else is tiny and runs on the host from the SAME quantized
    arrays: the numerator (exact f32), the 8 same-label exps and the
    first-50 hard-negative prefix (<= 66 columns per sample, fp8-dequant
    dots, consistent with the device quantization to ~1e-7).
  - Device output: one [128, 16] f32 tile per core (one column per piece).

Raw Bass (nc.Block) is used instead of the Tile framework: the installed
walrus rejects two raw-ISA instructions Tile's exit barrier emits."""

import numpy as np
import ml_dtypes

from contextlib import ExitStack

import concourse.bass as bass
from concourse import mybir
from concourse.bass_utils import run_bass_kernel_spmd

# problem constants (hardcoded per harness contract)
N, D, M = 256, 2048, 32000
L, C = 4000, 8
T = 0.07
LAMDA = 0.5
NCORES = 8
SHARD = M // NCORES          # 4000 centers per core
CAMW = SHARD // C            # 500 columns per camera per core
CAMP = 512                   # padded slab width (k-pair stride % 16 == 0)
KT = D // 128                # 16 k-tiles
NSLAB = 4                    # slab ring depth
NPSUM = 4                    # psum ring depth
NWARM = 16                   # dummy matmuls to warm the PE clock gate
ACCW = 16                    # fixed accumulator width (>= max piece count)

F32 = mybir.dt.float32
FP8 = mybir.dt.float8e4
DR = mybir.MatmulPerfMode.DoubleRow
CSCALE = 32.0                # host scales centers by 32 before fp8 cast
EXP = mybir.ActivationFunctionType.Exp


def _schedule(counts):
    """chunks: cams with samples; pieces[i]: list of (p0, p1) row ranges
    (<=128 wide) of permuted samples for chunk i."""
    offs = np.concatenate([[0], np.cumsum(counts)]).astype(int)
    chunks = [g for g in range(C) if counts[g] > 0]
    pieces = []
    for g in chunks:
        r0, r1 = int(offs[g]), int(offs[g + 1])
        cuts = list(range(r0, r1, 128)) + [r1]
        pieces.append([(cuts[i], cuts[i + 1]) for i in range(len(cuts) - 1)])
    return chunks, pieces


def _build_program(counts) -> bass.Bass:
    chunks, pieces = _schedule(counts)
    nch = len(chunks)
    cum = np.cumsum([len(p) for p in pieces]).astype(int)  # pieces thru chunk
    npieces = int(cum[-1])
    assert npieces <= ACCW

    nc = bass.Bass()
    ctg = nc.dram_tensor("ctg", [C, 128, KT, CAMP], FP8, kind="ExternalInput")
    fTp = nc.dram_tensor("fTp", [128, KT, N], FP8, kind="ExternalInput")
    svd = nc.dram_tensor("svd", [128, ACCW], F32, kind="ExternalInput")
    acc_out = nc.dram_tensor("ACC_out", [128, ACCW], F32, kind="ExternalOutput")

    with ExitStack() as ctx:
        e = ctx.enter_context

        ft_sb = e(nc.sbuf_tensor("ft_sb", [128, KT, N], FP8))
        slabs = [e(nc.sbuf_tensor(f"slab{j}", [128, KT, CAMP], FP8))
                 for j in range(NSLAB)]
        sv_sb = e(nc.sbuf_tensor("sv_sb", [128, ACCW], F32))
        scr = e(nc.sbuf_tensor("scr", [128, CAMW], F32))
        acc = e(nc.sbuf_tensor("acc", [128, ACCW], F32))

        ps = [e(nc.psum_tensor(f"ps{b}", [128, CAMP], F32)) for b in range(NPSUM)]

        sem_ft = e(nc.semaphore("sem_ft"))
        sem_ftb = e(nc.semaphore("sem_ftb"))
        sem_slab = [e(nc.semaphore(f"sem_slab{j}")) for j in range(NSLAB)]
        sem_slab0b = e(nc.semaphore("sem_slab0b"))
        sem_sv = e(nc.semaphore("sem_sv"))
        sem_pe = e(nc.semaphore("sem_pe"))
        sem_act = e(nc.semaphore("sem_act"))
        sem_od = e(nc.semaphore("sem_od"))

        block = e(nc.Block(no_gpsimd_drain=True))

        @block.sync
        def _(sync):
            # minimal path to the first matmul: k-halves of feats + slab 0
            g0 = chunks[0]
            sync.dma_start(out=ft_sb[:, 0:8, :], in_=fTp[:, 0:8, :]).then_inc(
                sem_ft, 16)
            sync.dma_start(out=slabs[0][:, 0:8, :],
                           in_=ctg[g0, :, 0:8, :]).then_inc(sem_slab[0], 16)
            sync.dma_start(out=ft_sb[:, 8:16, :], in_=fTp[:, 8:16, :]).then_inc(
                sem_ftb, 16)
            sync.dma_start(out=slabs[0][:, 8:16, :],
                           in_=ctg[g0, :, 8:16, :]).then_inc(sem_slab0b, 16)
            for idx in range(1, nch):
                j = idx % NSLAB
                if idx >= NSLAB:
                    # slot free once PE finished chunk idx-NSLAB
                    sync.wait_ge(sem_pe, int(cum[idx - NSLAB]))
                sync.dma_start(out=slabs[j][:, :, :],
                               in_=ctg[chunks[idx]]).then_inc(sem_slab[j], 16)
            sync.wait_ge(sem_act, npieces)
            sync.dma_start(out=acc_out[:, :], in_=acc[:, :]).then_inc(sem_od, 16)
            sync.wait_ge(sem_od, 16)

        @block.tensor
        def _(tensor):
            tensor.wait_ge(sem_ft, 16)
            # dummy matmuls on the already-loaded ft half: warms the PE clock
            # gate (HAM) while the first center slab is still in flight
            for w in range(NWARM):
                tensor.matmul(ps[NPSUM - 1][:, 0:128], ft_sb[:, 0:2, 0:128],
                              ft_sb[:, 0:2, 0:128], start=True, stop=True,
                              perf_mode=DR)
            slot_seen = [0] * NSLAB
            pc = 0                          # global piece counter
            for idx in range(nch):
                j = idx % NSLAB
                if idx == 0:
                    tensor.wait_ge(sem_slab[0], 16)   # first k-half only
                    slot_seen[0] = 16
                else:
                    slot_seen[j] += 16
                    tensor.wait_ge(sem_slab[j], slot_seen[j])
                for pi, (p0, p1) in enumerate(pieces[idx]):
                    b = pc % NPSUM
                    if pc >= NPSUM:
                        # psum slot free once ACT consumed piece pc-NPSUM
                        tensor.wait_ge(sem_act, pc - NPSUM + 1)
                    for ki in range(0, KT, 2):
                        if idx == 0 and pi == 0 and ki == 8:
                            tensor.wait_ge(sem_ftb, 16)
                            tensor.wait_ge(sem_slab0b, 16)
                        last = tensor.matmul(
                            ps[b][0:p1 - p0, 0:CAMP],
                            ft_sb[:, ki:ki + 2, p0:p1],
                            slabs[j][:, ki:ki + 2, 0:CAMP],
                            start=(ki == 0), stop=(ki == KT - 2),
                            perf_mode=DR)
                    last.then_inc(sem_pe, 1)
                    pc += 1

        @block.scalar
        def _(scalar):
            # per-sample exp scale rides the ACT engine's own HW-DGE ring
            scalar.dma_start(out=sv_sb[:, :], in_=svd[:, :]).then_inc(sem_sv, 16)
            scalar.wait_ge(sem_sv, 16)
            # exp straight out of PSUM; fused accum_out produces the
            # per-sample partial intra denominator for this camera slab
            pc = 0
            for idx in range(nch):
                for (p0, p1) in pieces[idx]:
                    n = p1 - p0
                    scalar.wait_ge(sem_pe, pc + 1)
                    scalar.activation(
                        out=scr[0:n, 0:CAMW],
                        in_=ps[pc % NPSUM][0:n, 0:CAMW],
                        func=EXP, scale=sv_sb[0:n, pc:pc + 1],
                        accum_out=acc[0:n, pc:pc + 1]
                    ).then_inc(sem_act, 1)
                    pc += 1

    return nc


_PROGRAM_CACHE: dict[tuple, bass.Bass] = {}


def _program(counts) -> bass.Bass:
    key = tuple(int(x) for x in counts)
    if key not in _PROGRAM_CACHE:
        _PROGRAM_CACHE[key] = _build_program(counts)
    return _PROGRAM_CACHE[key]


F8 = ml_dtypes.float8_e4m3


def _make_in_maps(feats_p, centers, counts):
    # replicated: fp8 feats (transposed, k-tiled) + per-sample exp scales
    fT = np.ascontiguousarray(feats_p.T).astype(F8)     # [2048, 256]
    fTp = np.ascontiguousarray(
        fT.reshape(KT, 128, N).transpose(1, 0, 2))      # [128, 16, 256]
    fq = fT.astype(np.float32).T                        # dequantized [256, 2048]
    nrm8 = np.linalg.norm(fq, axis=1)                   # ||f8|| per sample
    sv = (1.0 / (CSCALE * T * nrm8)).astype(np.float32)
    _, pieces = _schedule(counts)
    flat = [p for ch in pieces for p in ch]
    svd = np.zeros((128, ACCW), np.float32)
    for q, (p0, p1) in enumerate(flat):
        svd[0:p1 - p0, q] = sv[p0:p1]

    cq = np.ascontiguousarray(centers.T * CSCALE).astype(F8)  # [2048, 32000]
    in_maps = []
    for c in range(NCORES):
        shard = cq[:, c * SHARD:(c + 1) * SHARD]        # [2048, 4000]
        # cam-major: [2048, 500, 8] -> per cam [128, KT, 512] (padded)
        ctg = np.zeros((C, 128, KT, CAMP), F8)
        by_cam = shard.reshape(D, CAMW, C)
        for g in range(C):
            cg = by_cam[:, :, g].reshape(KT, 128, CAMW).transpose(1, 0, 2)
            ctg[g, :, :, 0:CAMW] = cg
        in_maps.append({"ctg": ctg, "fTp": fTp, "svd": svd})
    return in_maps, fq, sv, flat


def _host_tail(results, fq, sv, flat, feats_p, centers, labels_p, camids_p,
               epoch):
    n = labels_p.shape[0]
    denom_intra = np.zeros(n, np.float32)
    accs = [r["ACC_out"] for r in results]
    for q, (p0, p1) in enumerate(flat):
        part = np.zeros(p1 - p0, np.float32)
        for a in accs:
            part += a[0:p1 - p0, q]
        denom_intra[p0:p1] = part

    # same-label exps + first-50 hard negatives, from the SAME quantized
    # arrays the device used (fp8-dequant f32 dots == PE fp8 matmul)
    def cq_cols(cols):
        return (centers[cols] * CSCALE).astype(F8).astype(np.float32)

    lbl_cols = (labels_p[:, None] * C + np.arange(C)[None, :]).reshape(-1)
    cql = cq_cols(lbl_cols).reshape(n, C, D)            # [n, 8, 2048]
    s_lbl = np.einsum('nrd,nd->nr', cql, fq) * sv[:, None]
    B = np.exp(s_lbl).sum(axis=1)
    cqh = cq_cols(np.arange(58))                        # [58, 2048]
    s_head = (fq @ cqh.T) * sv[:, None]
    eh = np.exp(s_head)
    p50 = eh[:, 0:50].sum(axis=1)
    p58 = eh[:, 0:58].sum(axis=1)
    hard = np.where(labels_p <= 6, p58 - B, p50)
    denom_inter = B + hard

    # exact f32 numerator
    own_centers = centers[labels_p * C + camids_p]
    nrm = np.linalg.norm(feats_p, axis=1)
    own = np.einsum('nd,nd->n', feats_p, own_centers) / (T * nrm)

    loss_i = own - np.log(denom_intra)
    loss_j = own - np.log(denom_inter)

    cam_sums = np.zeros(C, np.float32)
    cam_cnts = np.zeros(C, np.float32)
    np.add.at(cam_sums, camids_p, loss_i)
    np.add.at(cam_cnts, camids_p, 1.0)
    loss_intra = -np.sum(
        np.where(cam_cnts > 0, cam_sums / np.maximum(cam_cnts, 1.0), 0.0),
        dtype=np.float32)

    lbl_sums = np.zeros(L, np.float32)
    lbl_cnts = np.zeros(L, np.float32)
    np.add.at(lbl_sums, labels_p, loss_j)
    np.add.at(lbl_cnts, labels_p, 1.0)
    loss_inter = -np.sum(
        np.where(lbl_cnts > 0, lbl_sums / np.maximum(lbl_cnts, 1.0), 0.0),
        dtype=np.float32)

    if int(epoch) < 5:
        return np.float32(loss_intra)
    return np.stack([loss_intra, LAMDA * loss_inter]).astype(np.float32)


def kernel(feats, centers, labels, camids, epoch):
    feats = np.ascontiguousarray(np.asarray(feats, dtype=np.float32))
    centers = np.ascontiguousarray(np.asarray(centers, dtype=np.float32))
    labels = np.asarray(labels).astype(np.int64)
    camids = np.asarray(camids).astype(np.int64)

    perm = np.argsort(camids, kind="stable")
    feats_p, labels_p, camids_p = feats[perm], labels[perm], camids[perm]
    counts = np.bincount(camids_p, minlength=C)

    in_maps, fq, sv, flat = _make_in_maps(feats_p, centers, counts)
    res = run_bass_kernel_spmd(_program(counts), in_maps,
                               list(range(NCORES))).results
    return _host_tail(res, fq, sv, flat, feats_p, centers, labels_p,
                      camids_p, epoch)


# revision 17
# speedup vs baseline: 2.3418x; 1.0388x over previous
"""Trainium2 Bass kernel for the CAP loss (camera-aware proxy memory bank).

Strategy (8 NeuronCores, SPMD, raw Bass engine blocks):
  - The center bank [32000, 2048] is sharded along the center axis (4000
    centers per core) and reordered cam-major on the host: each core holds
    8 slabs of 500 columns (one slab per camera), fp8(e4m3), scaled x32,
    pre-transposed to [128, 16, 512] (cols padded 500->512 for the
    DoubleRow k-pair stride requirement).
  - Samples are sorted by camid on the host; feats are replicated (fp8).
    Per slab g the PE computes only the rows of camera g (DoubleRow fp8
    matmuls, K=2048 accumulated in PSUM) - the intra-camera mask reduces
    useful compute 8x vs the dense [256 x 4000] product. Outputs land at
    PSUM partition base 0; the per-sample exp scale and the accumulator
    are laid out per piece (one column each), so no partition alignment
    with the sample index is needed.
  - The ACT engine applies exp straight out of PSUM with a per-sample
    1/(32*T*||f8||) scale and its fused accum_out produces the per-sample
    partial intra denominators directly. No vector-engine work at all.
  - Everything else is tiny and runs on the host from the SAME quantized
    arrays: the numerator (exact f32), the 8 same-label exps and the
    first-50 hard-negative prefix (<= 66 columns per sample, fp8-dequant
    dots, consistent with the device quantization to ~1e-7).
  - Device output: one [128, 16] f32 tile per core (one column per piece).

Raw Bass (nc.Block) is used instead of the Tile framework: the installed
walrus rejects two raw-ISA instructions Tile's exit barrier emits."""

import numpy as np
import ml_dtypes

from contextlib import ExitStack, contextmanager

import concourse.bass as bass
from concourse import mybir
from concourse.bass_utils import run_bass_kernel_spmd

# problem constants (hardcoded per harness contract)
N, D, M = 256, 2048, 32000
L, C = 4000, 8
T = 0.07
LAMDA = 0.5
NCORES = 8
SHARD = M // NCORES          # 4000 centers per core
CAMW = SHARD // C            # 500 columns per camera per core
CAMP = 512                   # padded slab width (k-pair stride % 16 == 0)
KT = D // 128                # 16 k-tiles
NSLAB = 4                    # slab ring depth
NPSUM = 4                    # psum ring depth
NWARM = 16                   # dummy matmuls to warm the PE clock gate
ACCW = 16                    # fixed accumulator width (>= max piece count)

F32 = mybir.dt.float32
FP8 = mybir.dt.float8e4
DR = mybir.MatmulPerfMode.DoubleRow
CSCALE = 32.0                # host scales centers by 32 before fp8 cast
EXP = mybir.ActivationFunctionType.Exp


@contextmanager
def _lean_block(nc):
    """nc.Block without the end-of-program all-engine event-semaphore
    barrier (~9us of counted epilogue): engines just branch to the end
    block and drain; the runtime completes when every queue retires."""
    nc.check_frozen()
    assert nc.cur_block is None
    blk = bass.BassBlock(nc, f"block_{nc.next_id()}", no_gpsimd_drain=True)
    nc.cur_block = blk
    yield blk
    for engine, last_body in blk.last_body.items():
        with nc.body(last_body, parent=nc.cur_bb, allow_existing_parent=True):
            engine.br(blk.end_bb)
    nc.switch_bb(blk.end_bb)
    gpsimd_type = nc.gpsimd.engine
    for eng_type, eng in nc.engines.items():
        if eng_type == gpsimd_type:
            continue
        d = mybir.InstDrain(
            name=nc.get_next_instruction_name(),
            ins=[], outs=[], bass_is_fusable=False,
        )
        d.engine = eng_type
        eng.add_instruction(d)
    nc.cur_block = None


def _schedule(counts):
    """chunks: cams with samples; pieces[i]: list of (p0, p1) row ranges
    (<=128 wide) of permuted samples for chunk i."""
    offs = np.concatenate([[0], np.cumsum(counts)]).astype(int)
    chunks = [g for g in range(C) if counts[g] > 0]
    pieces = []
    for g in chunks:
        r0, r1 = int(offs[g]), int(offs[g + 1])
        cuts = list(range(r0, r1, 128)) + [r1]
        pieces.append([(cuts[i], cuts[i + 1]) for i in range(len(cuts) - 1)])
    return chunks, pieces


def _build_program(counts) -> bass.Bass:
    chunks, pieces = _schedule(counts)
    nch = len(chunks)
    cum = np.cumsum([len(p) for p in pieces]).astype(int)  # pieces thru chunk
    npieces = int(cum[-1])
    assert npieces <= ACCW

    nc = bass.Bass()
    ctg = nc.dram_tensor("ctg", [C, 128, KT, CAMP], FP8, kind="ExternalInput")
    fTp = nc.dram_tensor("fTp", [128, KT, N], FP8, kind="ExternalInput")
    svd = nc.dram_tensor("svd", [128, ACCW], F32, kind="ExternalInput")
    acc_out = nc.dram_tensor("ACC_out", [128, ACCW], F32, kind="ExternalOutput")

    with ExitStack() as ctx:
        e = ctx.enter_context

        ft_sb = e(nc.sbuf_tensor("ft_sb", [128, KT, N], FP8))
        slabs = [e(nc.sbuf_tensor(f"slab{j}", [128, KT, CAMP], FP8))
                 for j in range(NSLAB)]
        sv_sb = e(nc.sbuf_tensor("sv_sb", [128, ACCW], F32))
        scr = e(nc.sbuf_tensor("scr", [128, CAMW], F32))
        acc = e(nc.sbuf_tensor("acc", [128, ACCW], F32))

        ps = [e(nc.psum_tensor(f"ps{b}", [128, CAMP], F32)) for b in range(NPSUM)]

        sem_ft = e(nc.semaphore("sem_ft"))
        sem_ftb = e(nc.semaphore("sem_ftb"))
        sem_slab = [e(nc.semaphore(f"sem_slab{j}")) for j in range(NSLAB)]
        sem_slab0b = e(nc.semaphore("sem_slab0b"))
        sem_sv = e(nc.semaphore("sem_sv"))
        sem_pe = e(nc.semaphore("sem_pe"))
        sem_act = e(nc.semaphore("sem_act"))
        sem_od = e(nc.semaphore("sem_od"))

        block = e(_lean_block(nc))

        @block.sync
        def _(sync):
            # minimal path to the first matmul: k-halves of feats + slab 0
            g0 = chunks[0]
            sync.dma_start(out=ft_sb[:, 0:8, :], in_=fTp[:, 0:8, :]).then_inc(
                sem_ft, 16)
            sync.dma_start(out=slabs[0][:, 0:8, :],
                           in_=ctg[g0, :, 0:8, :]).then_inc(sem_slab[0], 16)
            sync.dma_start(out=ft_sb[:, 8:16, :], in_=fTp[:, 8:16, :]).then_inc(
                sem_ftb, 16)
            sync.dma_start(out=slabs[0][:, 8:16, :],
                           in_=ctg[g0, :, 8:16, :]).then_inc(sem_slab0b, 16)
            for idx in range(1, nch):
                j = idx % NSLAB
                if idx >= NSLAB:
                    # slot free once PE finished chunk idx-NSLAB
                    sync.wait_ge(sem_pe, int(cum[idx - NSLAB]))
                sync.dma_start(out=slabs[j][:, :, :],
                               in_=ctg[chunks[idx]]).then_inc(sem_slab[j], 16)
            sync.wait_ge(sem_act, npieces)
            sync.dma_start(out=acc_out[:, :], in_=acc[:, :]).then_inc(sem_od, 16)
            sync.wait_ge(sem_od, 16)

        @block.tensor
        def _(tensor):
            tensor.wait_ge(sem_ft, 16)
            # dummy matmuls on the already-loaded ft half: warms the PE clock
            # gate (HAM) while the first center slab is still in flight
            for w in range(NWARM):
                tensor.matmul(ps[NPSUM - 1][:, 0:128], ft_sb[:, 0:2, 0:128],
                              ft_sb[:, 0:2, 0:128], start=True, stop=True,
                              perf_mode=DR)
            slot_seen = [0] * NSLAB
            pc = 0                          # global piece counter
            for idx in range(nch):
                j = idx % NSLAB
                if idx == 0:
                    tensor.wait_ge(sem_slab[0], 16)   # first k-half only
                    slot_seen[0] = 16
                else:
                    slot_seen[j] += 16
                    tensor.wait_ge(sem_slab[j], slot_seen[j])
                for pi, (p0, p1) in enumerate(pieces[idx]):
                    b = pc % NPSUM
                    if pc >= NPSUM:
                        # psum slot free once ACT consumed piece pc-NPSUM
                        tensor.wait_ge(sem_act, pc - NPSUM + 1)
                    for ki in range(0, KT, 2):
                        if idx == 0 and pi == 0 and ki == 8:
                            tensor.wait_ge(sem_ftb, 16)
                            tensor.wait_ge(sem_slab0b, 16)
                        last = tensor.matmul(
                            ps[b][0:p1 - p0, 0:CAMP],
                            ft_sb[:, ki:ki + 2, p0:p1],
                            slabs[j][:, ki:ki + 2, 0:CAMP],
                            start=(ki == 0), stop=(ki == KT - 2),
                            perf_mode=DR)
                    last.then_inc(sem_pe, 1)
                    pc += 1

        @block.scalar
        def _(scalar):
            # per-sample exp scale rides the ACT engine's own HW-DGE ring
            scalar.dma_start(out=sv_sb[:, :], in_=svd[:, :]).then_inc(sem_sv, 16)
            scalar.wait_ge(sem_sv, 16)
            # exp straight out of PSUM; fused accum_out produces the
            # per-sample partial intra denominator for this camera slab
            pc = 0
            for idx in range(nch):
                for (p0, p1) in pieces[idx]:
                    n = p1 - p0
                    scalar.wait_ge(sem_pe, pc + 1)
                    scalar.activation(
                        out=scr[0:n, 0:CAMW],
                        in_=ps[pc % NPSUM][0:n, 0:CAMW],
                        func=EXP, scale=sv_sb[0:n, pc:pc + 1],
                        accum_out=acc[0:n, pc:pc + 1]
                    ).then_inc(sem_act, 1)
                    pc += 1

    return nc


_PROGRAM_CACHE: dict[tuple, bass.Bass] = {}


def _program(counts) -> bass.Bass:
    key = tuple(int(x) for x in counts)
    if key not in _PROGRAM_CACHE:
        _PROGRAM_CACHE[key] = _build_program(counts)
    return _PROGRAM_CACHE[key]


F8 = ml_dtypes.float8_e4m3


def _make_in_maps(feats_p, centers, counts):
    # replicated: fp8 feats (transposed, k-tiled) + per-sample exp scales
    fT = np.ascontiguousarray(feats_p.T).astype(F8)     # [2048, 256]
    fTp = np.ascontiguousarray(
        fT.reshape(KT, 128, N).transpose(1, 0, 2))      # [128, 16, 256]
    fq = fT.astype(np.float32).T                        # dequantized [256, 2048]
    nrm8 = np.linalg.norm(fq, axis=1)                   # ||f8|| per sample
    sv = (1.0 / (CSCALE * T * nrm8)).astype(np.float32)
    _, pieces = _schedule(counts)
    flat = [p for ch in pieces for p in ch]
    svd = np.zeros((128, ACCW), np.float32)
    for q, (p0, p1) in enumerate(flat):
        svd[0:p1 - p0, q] = sv[p0:p1]

    cq = np.ascontiguousarray(centers.T * CSCALE).astype(F8)  # [2048, 32000]
    in_maps = []
    for c in range(NCORES):
        shard = cq[:, c * SHARD:(c + 1) * SHARD]        # [2048, 4000]
        # cam-major: [2048, 500, 8] -> per cam [128, KT, 512] (padded)
        ctg = np.zeros((C, 128, KT, CAMP), F8)
        by_cam = shard.reshape(D, CAMW, C)
        for g in range(C):
            cg = by_cam[:, :, g].reshape(KT, 128, CAMW).transpose(1, 0, 2)
            ctg[g, :, :, 0:CAMW] = cg
        in_maps.append({"ctg": ctg, "fTp": fTp, "svd": svd})
    return in_maps, fq, sv, flat


def _host_tail(results, fq, sv, flat, feats_p, centers, labels_p, camids_p,
               epoch):
    n = labels_p.shape[0]
    denom_intra = np.zeros(n, np.float32)
    accs = [r["ACC_out"] for r in results]
    for q, (p0, p1) in enumerate(flat):
        part = np.zeros(p1 - p0, np.float32)
        for a in accs:
            part += a[0:p1 - p0, q]
        denom_intra[p0:p1] = part

    # same-label exps + first-50 hard negatives, from the SAME quantized
    # arrays the device used (fp8-dequant f32 dots == PE fp8 matmul)
    def cq_cols(cols):
        return (centers[cols] * CSCALE).astype(F8).astype(np.float32)

    lbl_cols = (labels_p[:, None] * C + np.arange(C)[None, :]).reshape(-1)
    cql = cq_cols(lbl_cols).reshape(n, C, D)            # [n, 8, 2048]
    s_lbl = np.einsum('nrd,nd->nr', cql, fq) * sv[:, None]
    B = np.exp(s_lbl).sum(axis=1)
    cqh = cq_cols(np.arange(58))                        # [58, 2048]
    s_head = (fq @ cqh.T) * sv[:, None]
    eh = np.exp(s_head)
    p50 = eh[:, 0:50].sum(axis=1)
    p58 = eh[:, 0:58].sum(axis=1)
    hard = np.where(labels_p <= 6, p58 - B, p50)
    denom_inter = B + hard

    # exact f32 numerator
    own_centers = centers[labels_p * C + camids_p]
    nrm = np.linalg.norm(feats_p, axis=1)
    own = np.einsum('nd,nd->n', feats_p, own_centers) / (T * nrm)

    loss_i = own - np.log(denom_intra)
    loss_j = own - np.log(denom_inter)

    cam_sums = np.zeros(C, np.float32)
    cam_cnts = np.zeros(C, np.float32)
    np.add.at(cam_sums, camids_p, loss_i)
    np.add.at(cam_cnts, camids_p, 1.0)
    loss_intra = -np.sum(
        np.where(cam_cnts > 0, cam_sums / np.maximum(cam_cnts, 1.0), 0.0),
        dtype=np.float32)

    lbl_sums = np.zeros(L, np.float32)
    lbl_cnts = np.zeros(L, np.float32)
    np.add.at(lbl_sums, labels_p, loss_j)
    np.add.at(lbl_cnts, labels_p, 1.0)
    loss_inter = -np.sum(
        np.where(lbl_cnts > 0, lbl_sums / np.maximum(lbl_cnts, 1.0), 0.0),
        dtype=np.float32)

    if int(epoch) < 5:
        return np.float32(loss_intra)
    return np.stack([loss_intra, LAMDA * loss_inter]).astype(np.float32)


def kernel(feats, centers, labels, camids, epoch):
    feats = np.ascontiguousarray(np.asarray(feats, dtype=np.float32))
    centers = np.ascontiguousarray(np.asarray(centers, dtype=np.float32))
    labels = np.asarray(labels).astype(np.int64)
    camids = np.asarray(camids).astype(np.int64)

    perm = np.argsort(camids, kind="stable")
    feats_p, labels_p, camids_p = feats[perm], labels[perm], camids[perm]
    counts = np.bincount(camids_p, minlength=C)

    in_maps, fq, sv, flat = _make_in_maps(feats_p, centers, counts)
    res = run_bass_kernel_spmd(_program(counts), in_maps,
                               list(range(NCORES))).results
    return _host_tail(res, fq, sv, flat, feats_p, centers, labels_p,
                      camids_p, epoch)
